# revision 10
# baseline (speedup 1.0000x reference)
"""Trainium2 Bass kernel for nn_AnalysisLayer (histogram_binning).

reference:
    channel_mean_abs = mean(|data_in|, axis=(0,2,3))   # [C]
    new_energy = td_energy_3d + channel_mean_abs
    new_hist   = td_hist + 1
    return data_in, new_energy, new_hist

Strategy (data-parallel over batch, 8 cores):
    data_in is [32, 256, 64, 64] f32 (128 MiB).  Core k takes batches
    4k..4k+4 (16 MiB), views them as [1024, 4096] ( (b,c) rows x (h,w) ),
    streams 8 tiles of [128ch, 4096] into SBUF, and for each tile does a
    single fused abs+sum on the vector engine
    (tensor_reduce(op=add, apply_absolute_value=True)) producing [128,1]
    partial sums.  One strided reduce combines the 8 per-tile partials into
    [128, 2] per-core channel sums, DMA'd out as [128,2].  Host gathers the
    8 partials, sums, divides by B*H*W, and applies the trivial adds.

Raw Bass (not Tile): the whole 16 MiB fits in SBUF at once, so all 8 load
DMAs are issued up front with one completion semaphore each, and the vector
engine walks the tiles in order.  No slot reuse -> no WAR hazards.
"""

import sys
from contextlib import ExitStack

for _p in ("/opt/trn_rl_repo", "/opt/pypackages"):
    if _p not in sys.path:
        sys.path.append(_p)

import numpy as np

import concourse.bass as bass
import concourse.mybir as mybir
from concourse.bass_utils import run_bass_kernel_spmd

N_CORES = 8
B, C, H, W = 32, 256, 64, 64
B_LOC = B // N_CORES          # 4 batches per core
ROWS = B_LOC * C              # 1024 (b, c) rows per core
FREE = H * W                  # 4096
P = 128                       # partitions
NT = ROWS // P                # 8 tiles per core
HALVES = C // P               # 2 channel halves


def _build_nc() -> bass.Bass:
    nc = bass.Bass()
    x = nc.dram_tensor("x", [ROWS, FREE], mybir.dt.float32, kind="ExternalInput")
    out = nc.dram_tensor("out", [P, HALVES], mybir.dt.float32, kind="ExternalOutput")

    with (
        nc.sbuf_tensor([P, NT * FREE], mybir.dt.float32) as data,
        nc.sbuf_tensor([P, NT], mybir.dt.float32) as stats,
        nc.sbuf_tensor([P, HALVES], mybir.dt.float32) as res,
        ExitStack() as sem_ctx,
        nc.Block() as block,
    ):
        dsems = [sem_ctx.enter_context(nc.semaphore(f"dsem{i}")) for i in range(NT)]
        vsem = sem_ctx.enter_context(nc.semaphore("vsem"))
        osem = sem_ctx.enter_context(nc.semaphore("osem"))

        @block.sync
        def _(sync):
            for i in range(NT):
                sync.dma_start(
                    out=data[:, i * FREE : (i + 1) * FREE],
                    in_=x[i * P : (i + 1) * P, :],
                ).then_inc(dsems[i], 16)

        @block.vector
        def _(vector):
            for i in range(NT):
                vector.wait_ge(dsems[i], 16)
                vector.tensor_reduce(
                    out=stats[:, i : i + 1],
                    in_=data[:, i * FREE : (i + 1) * FREE],
                    axis=mybir.AxisListType.X,
                    op=mybir.AluOpType.add,
                    apply_absolute_value=True,
                )
            # Column i of stats = (batch i//2, half i%2).  View as
            # [P, half, batch] and reduce the innermost (batch) axis.
            stats_view = stats[:, :].rearrange("p (b h) -> p h b", h=HALVES)
            vector.tensor_reduce(
                out=res[:, :],
                in_=stats_view,
                axis=mybir.AxisListType.X,
                op=mybir.AluOpType.add,
            ).then_inc(vsem, 1)

        @block.gpsimd
        def _(gpsimd):
            gpsimd.wait_ge(vsem, 1)
            gpsimd.dma_start(out=out[:, :], in_=res[:, :]).then_inc(osem, 16)
            gpsimd.wait_ge(osem, 16)

    return nc


_NC_CACHE = None


def kernel(data_in, td_energy_3d, td_hist):
    global _NC_CACHE
    data_in = np.asarray(data_in, dtype=np.float32)
    td_energy_3d = np.asarray(td_energy_3d, dtype=np.float32)
    td_hist = np.asarray(td_hist, dtype=np.float32)

    if _NC_CACHE is None:
        _NC_CACHE = _build_nc()
    nc = _NC_CACHE

    shards = data_in.reshape(N_CORES, ROWS, FREE)
    in_maps = [{"x": shards[k]} for k in range(N_CORES)]
    results = run_bass_kernel_spmd(nc, in_maps, list(range(N_CORES))).results

    # results[k]["out"] is [128, 2]: out[p, h] = abs-sum of channel h*128+p
    total = np.zeros((C,), dtype=np.float32)
    for r in results:
        total += np.asarray(r["out"], dtype=np.float32).T.reshape(C)
    channel_mean_abs = total / np.float32(B * H * W)

    new_energy = td_energy_3d + channel_mean_abs
    new_hist = td_hist + np.float32(1.0)
    return data_in, new_energy, new_hist


# revision 13
# speedup vs baseline: 1.0498x; 1.0498x over previous
"""Trainium2 Bass kernel for nn_AnalysisLayer (histogram_binning).

reference:
    channel_mean_abs = mean(|data_in|, axis=(0,2,3))   # [C]
    new_energy = td_energy_3d + channel_mean_abs
    new_hist   = td_hist + 1
    return data_in, new_energy, new_hist

Strategy (data-parallel over batch, 8 cores):
    data_in is [32, 256, 64, 64] f32 (128 MiB).  Core k takes batches
    4k..4k+4 (16 MiB), views them as [1024, 4096] ( (b,c) rows x (h,w) ),
    streams 8 tiles of [128ch, 4096] into SBUF, and for each tile does a
    single fused abs+sum on the vector engine
    (tensor_reduce(op=add, apply_absolute_value=True)) producing [128,1]
    partial sums.  One strided reduce combines the 8 per-tile partials into
    [128, 2] per-core channel sums, DMA'd out as [128,2].  Host gathers the
    8 partials, sums, divides by B*H*W, and applies the trivial adds.

Raw Bass (not Tile): the whole 16 MiB fits in SBUF at once, so all 8 load
DMAs are issued up front with one completion semaphore each, and the vector
engine walks the tiles in order.  No slot reuse -> no WAR hazards.
"""

import sys
from contextlib import ExitStack

for _p in ("/opt/trn_rl_repo", "/opt/pypackages"):
    if _p not in sys.path:
        sys.path.append(_p)

import numpy as np

import concourse.bass as bass
import concourse.mybir as mybir
from concourse.bass_utils import run_bass_kernel_spmd

N_CORES = 8
B, C, H, W = 32, 256, 64, 64
B_LOC = B // N_CORES          # 4 batches per core
ROWS = B_LOC * C              # 1024 (b, c) rows per core
FREE = H * W                  # 4096
P = 128                       # partitions
NT = ROWS // P                # 8 tiles per core
HALVES = C // P               # 2 channel halves


def _build_nc() -> bass.Bass:
    nc = bass.Bass(enable_partition_id=False)
    x = nc.dram_tensor("x", [ROWS, FREE], mybir.dt.float32, kind="ExternalInput")
    out = nc.dram_tensor("out", [P, HALVES], mybir.dt.float32, kind="ExternalOutput")

    # Per-tile column split between the two reduce-capable engines:
    # DVE tensor_reduce(abs) runs ~1.04 ns/col, ACT activation(Abs)+accum
    # ~0.83 ns/col; split so both finish together.
    DVE_COLS = 1824
    ACT_COLS = FREE - DVE_COLS

    with (
        nc.sbuf_tensor([P, NT * FREE], mybir.dt.float32) as data,
        nc.sbuf_tensor([P, 2 * NT], mybir.dt.float32) as stats,
        nc.sbuf_tensor([P, HALVES], mybir.dt.float32) as res,
        ExitStack() as sem_ctx,
        nc.Block() as block,
    ):
        # One completion sem for all loads: SP-issued HWDGE DMAs share one
        # ring (qSPDynamicHW) and drain FIFO per SDMA engine, so sem >=
        # 16*(i+1) implies tiles 0..i have fully landed.
        dsem = sem_ctx.enter_context(nc.semaphore("dsem"))
        asem = sem_ctx.enter_context(nc.semaphore("asem"))
        vsem = sem_ctx.enter_context(nc.semaphore("vsem"))
        osem = sem_ctx.enter_context(nc.semaphore("osem"))

        @block.sync
        def _(sync):
            for i in range(NT):
                sync.dma_start(
                    out=data[:, i * FREE : (i + 1) * FREE],
                    in_=x[i * P : (i + 1) * P, :],
                ).then_inc(dsem, 16)
            sync.wait_ge(vsem, 1)
            sync.dma_start(out=out[:, :], in_=res[:, :]).then_inc(osem, 16)
            sync.wait_ge(osem, 16)

        @block.vector
        def _(vector):
            for i in range(NT):
                vector.wait_ge(dsem, 16 * (i + 1))
                vector.tensor_reduce(
                    out=stats[:, 2 * i : 2 * i + 1],
                    in_=data[:, i * FREE : i * FREE + DVE_COLS],
                    axis=mybir.AxisListType.X,
                    op=mybir.AluOpType.add,
                    apply_absolute_value=True,
                )
            # Column 2i   of stats = DVE partial for tile i,
            # column 2i+1 of stats = ACT partial for tile i;
            # tile i = (batch i//2, half i%2) -> col = b*4 + h*2 + e.
            # View as [P, h, b, e] and reduce the two innermost axes.
            stats_view = stats[:, :].rearrange(
                "p (b h e) -> p h b e", h=HALVES, e=2
            )
            vector.wait_ge(asem, NT)
            vector.tensor_reduce(
                out=res[:, :],
                in_=stats_view,
                axis=mybir.AxisListType.XY,
                op=mybir.AluOpType.add,
            ).then_inc(vsem, 1)

        @block.scalar
        def _(scalar):
            for i in range(NT):
                scalar.wait_ge(dsem, 16 * (i + 1))
                scalar.activation(
                    out=data[:, i * FREE + DVE_COLS : (i + 1) * FREE],
                    in_=data[:, i * FREE + DVE_COLS : (i + 1) * FREE],
                    func=mybir.ActivationFunctionType.Abs,
                    accum_out=stats[:, 2 * i + 1 : 2 * i + 2],
                ).then_inc(asem, 1)

    return nc


_NC_CACHE = None


def kernel(data_in, td_energy_3d, td_hist):
    global _NC_CACHE
    data_in = np.asarray(data_in, dtype=np.float32)
    td_energy_3d = np.asarray(td_energy_3d, dtype=np.float32)
    td_hist = np.asarray(td_hist, dtype=np.float32)

    if _NC_CACHE is None:
        _NC_CACHE = _build_nc()
    nc = _NC_CACHE

    shards = data_in.reshape(N_CORES, ROWS, FREE)
    in_maps = [{"x": shards[k]} for k in range(N_CORES)]
    results = run_bass_kernel_spmd(nc, in_maps, list(range(N_CORES))).results

    # results[k]["out"] is [128, 2]: out[p, h] = abs-sum of channel h*128+p
    total = np.zeros((C,), dtype=np.float32)
    for r in results:
        total += np.asarray(r["out"], dtype=np.float32).T.reshape(C)
    channel_mean_abs = total / np.float32(B * H * W)

    new_energy = td_energy_3d + channel_mean_abs
    new_hist = td_hist + np.float32(1.0)
    return data_in, new_energy, new_hist


# revision 14
# speedup vs baseline: 1.0780x; 1.0269x over previous
"""Trainium2 Bass kernel for nn_AnalysisLayer (histogram_binning).

reference:
    channel_mean_abs = mean(|data_in|, axis=(0,2,3))   # [C]
    new_energy = td_energy_3d + channel_mean_abs
    new_hist   = td_hist + 1
    return data_in, new_energy, new_hist

Strategy (data-parallel over batch, 8 cores):
    data_in is [32, 256, 64, 64] f32 (128 MiB).  Core k takes batches
    4k..4k+4 (16 MiB), views them as [1024, 4096] ( (b,c) rows x (h,w) ),
    streams 8 tiles of [128ch, 4096] into SBUF, and for each tile does a
    single fused abs+sum on the vector engine
    (tensor_reduce(op=add, apply_absolute_value=True)) producing [128,1]
    partial sums.  One strided reduce combines the 8 per-tile partials into
    [128, 2] per-core channel sums, DMA'd out as [128,2].  Host gathers the
    8 partials, sums, divides by B*H*W, and applies the trivial adds.

Raw Bass (not Tile): the whole 16 MiB fits in SBUF at once, so all 8 load
DMAs are issued up front with one completion semaphore each, and the vector
engine walks the tiles in order.  No slot reuse -> no WAR hazards.
"""

import sys
from contextlib import ExitStack

for _p in ("/opt/trn_rl_repo", "/opt/pypackages"):
    if _p not in sys.path:
        sys.path.append(_p)

import numpy as np

import concourse.bass as bass
import concourse.mybir as mybir
from concourse.bass_utils import run_bass_kernel_spmd

N_CORES = 8
B, C, H, W = 32, 256, 64, 64
B_LOC = B // N_CORES          # 4 batches per core
ROWS = B_LOC * C              # 1024 (b, c) rows per core
FREE = H * W                  # 4096
P = 128                       # partitions
NT = ROWS // P                # 8 tiles per core
HALVES = C // P               # 2 channel halves


def _build_nc() -> bass.Bass:
    nc = bass.Bass(enable_partition_id=False)
    x = nc.dram_tensor("x", [ROWS, FREE], mybir.dt.float32, kind="ExternalInput")
    out = nc.dram_tensor("out", [P, HALVES], mybir.dt.float32, kind="ExternalOutput")

    # Load segments: (row_block, col_start, col_end).  Row blocks 0..6 are
    # loaded whole (2 MiB each); row block 7 is split so the final compute
    # bite after the last byte lands is small.
    SEGS = [(i, 0, FREE) for i in range(NT - 1)]
    SEGS += [(NT - 1, 0, 3072), (NT - 1, 3072, FREE)]

    # Per-segment column split between the two reduce-capable engines
    # (measured: DVE tensor_reduce(abs) 1.125 ns/col, ACT activation(Abs)
    # 1.085 ns/col + 278 ns accum-read): d such that both finish together.
    def _dve_cols(w: int) -> int:
        return min(w - 32, max(32, int(round((1.085 * w + 278) / 2.21 / 4)) * 4))

    # stats columns: half 0 partials in cols 0..7, half 1 in cols 8..17
    # (row block 7 contributes two segments -> 10 half-1 partials).
    seg_col = []
    next_col = {0: 0, 1: NT}
    for rb, c0, c1 in SEGS:
        h = rb % HALVES
        seg_col.append(next_col[h])
        next_col[h] += 2
    H0_COLS = NT          # cols [0, 8): half-0 partials
    NCOLS = next_col[1]   # 18

    with (
        nc.sbuf_tensor([P, NT * FREE], mybir.dt.float32) as data,
        nc.sbuf_tensor([P, NCOLS], mybir.dt.float32) as stats,
        nc.sbuf_tensor([P, HALVES], mybir.dt.float32) as res,
        ExitStack() as sem_ctx,
        nc.Block() as block,
    ):
        # One completion sem for all loads: SP-issued HWDGE DMAs share one
        # ring (qSPDynamicHW) and drain FIFO per SDMA engine, so sem >=
        # 16*(s+1) implies segments 0..s have fully landed.
        dsem = sem_ctx.enter_context(nc.semaphore("dsem"))
        asem = sem_ctx.enter_context(nc.semaphore("asem"))
        vsem = sem_ctx.enter_context(nc.semaphore("vsem"))
        osem = sem_ctx.enter_context(nc.semaphore("osem"))

        @block.sync
        def _(sync):
            for rb, c0, c1 in SEGS:
                sync.dma_start(
                    out=data[:, rb * FREE + c0 : rb * FREE + c1],
                    in_=x[rb * P : (rb + 1) * P, c0:c1],
                ).then_inc(dsem, 16)
            sync.wait_ge(vsem, 1)
            sync.dma_start(out=out[:, :], in_=res[:, :]).then_inc(osem, 16)
            sync.wait_ge(osem, 16)

        @block.vector
        def _(vector):
            for s, (rb, c0, c1) in enumerate(SEGS):
                d = _dve_cols(c1 - c0)
                vector.wait_ge(dsem, 16 * (s + 1))
                vector.tensor_reduce(
                    out=stats[:, seg_col[s] : seg_col[s] + 1],
                    in_=data[:, rb * FREE + c0 : rb * FREE + c0 + d],
                    axis=mybir.AxisListType.X,
                    op=mybir.AluOpType.add,
                    apply_absolute_value=True,
                )
            vector.wait_ge(asem, len(SEGS))
            vector.tensor_reduce(
                out=res[:, 0:1],
                in_=stats[:, 0:H0_COLS],
                axis=mybir.AxisListType.X,
                op=mybir.AluOpType.add,
            )
            vector.tensor_reduce(
                out=res[:, 1:2],
                in_=stats[:, H0_COLS:NCOLS],
                axis=mybir.AxisListType.X,
                op=mybir.AluOpType.add,
            ).then_inc(vsem, 1)

        @block.scalar
        def _(scalar):
            for s, (rb, c0, c1) in enumerate(SEGS):
                d = _dve_cols(c1 - c0)
                scalar.wait_ge(dsem, 16 * (s + 1))
                scalar.activation(
                    out=data[:, rb * FREE + c0 + d : rb * FREE + c1],
                    in_=data[:, rb * FREE + c0 + d : rb * FREE + c1],
                    func=mybir.ActivationFunctionType.Abs,
                    accum_out=stats[:, seg_col[s] + 1 : seg_col[s] + 2],
                ).then_inc(asem, 1)

    return nc


_NC_CACHE = None


def kernel(data_in, td_energy_3d, td_hist):
    global _NC_CACHE
    data_in = np.asarray(data_in, dtype=np.float32)
    td_energy_3d = np.asarray(td_energy_3d, dtype=np.float32)
    td_hist = np.asarray(td_hist, dtype=np.float32)

    if _NC_CACHE is None:
        _NC_CACHE = _build_nc()
    nc = _NC_CACHE

    shards = data_in.reshape(N_CORES, ROWS, FREE)
    in_maps = [{"x": shards[k]} for k in range(N_CORES)]
    results = run_bass_kernel_spmd(nc, in_maps, list(range(N_CORES))).results

    # results[k]["out"] is [128, 2]: out[p, h] = abs-sum of channel h*128+p
    total = np.zeros((C,), dtype=np.float32)
    for r in results:
        total += np.asarray(r["out"], dtype=np.float32).T.reshape(C)
    channel_mean_abs = total / np.float32(B * H * W)

    new_energy = td_energy_3d + channel_mean_abs
    new_hist = td_hist + np.float32(1.0)
    return data_in, new_energy, new_hist
